# revision 15
# baseline (speedup 1.0000x reference)
"""Trainium2 Bass kernel for nn_BatchedTrilLinear.

y[n, b*64:(b+1)*64] = x[n, b*64:(b+1)*64] @ L_b.T  for b in range(512),
with L_b = tril(W_b, -1) + diag(exp(diag(W_b))).

Sharding: data-parallel on N — each of the 8 cores processes a contiguous
512-row slice of x (zero-copy views) with the weights replicated.

Wire format: x and y cross HBM as bf16 (the matmul consumes bf16 anyway and
the 2e-2 gate dwarfs the bf16 output rounding), halving the dominant HBM
traffic. Weights ride as a host-side pure layout permutation of W:
  wt[64j+o, s, i] = W[2s+j, o, i]   (bf16, per-partition contiguous rows)
  wd[64j+o, s]    = W[2s+j, o, o]   (f32 diagonals for exact exp on device)
so the weight DMA uses large contiguous descriptors instead of 256 B strided
reads (which run ~5x below line rate). All math (tril mask, exp, transpose,
matmul) happens on device.

Per-core dataflow (natural-output matmul):
  - x loaded via SWDGE (gpsimd queue) in grouped HBM reads; y stored bf16
    via HWDGE on the SP queue; the two big DMA streams sit on different
    queues and overlap.
  - weights: per chunk, load wt/wd, mask strictly-lower via tril mask
    (gpsimd), exp(wd) on ACT, insert diag via mask-mult + add, then stage
    each block pair on the diagonal of a zeroed 128x128 tile and
    PE-transpose, giving the all-resident block-diagonal moving operands
    lt[64j+i, s, 64j'+o] = (j==j') * L_{2s+j}[o, i]  (bf16, 8 MB),
    interleaved chunk-by-chunk with the strip groups that consume them.
  - per strip (128 columns = 2 blocks b0=2s, b1=2s+1):
      4 PE transposes: x chunks [128 n,128 c] -> xT strip [128 (j,i), 512 n]
        in one half-bank bf16 PSUM tile; 1 copy to SBUF
      4 matmuls, one per 128-row n-tile: stationary = xT chunk [128,128],
        moving = lt[:, s, :] -> y tile lands in NATURAL [n, c] layout in
        PSUM; 1 copy (f32 PSUM -> bf16 SBUF group buffer)
      the two PSUM->SBUF copies alternate between DVE and ACT per strip so
      neither engine carries both copy streams.
"""
import os
import sys
from contextlib import ExitStack

for _p in ("/opt/trn_rl_repo",):
    if os.path.isdir(_p) and _p not in sys.path:
        sys.path.insert(0, _p)

import numpy as np

N_FULL = 4096
B_FULL = 512
D = 64
NCORES = 8
NS = N_FULL // NCORES        # rows per core

_built = {}


def _body(ctx, tc, y_d, x_d, wt_d, wd_d, *, NS, B, SG, SC, dt_name, repeat=1,
          y_dt=None):
    import concourse.mybir as mybir
    from concourse.masks import make_identity

    nc = tc.nc
    f32 = mybir.dt.float32
    dt = {"bf16": mybir.dt.bfloat16, "f32r": mybir.dt.float32r}[dt_name]
    if y_dt is None:
        y_dt = f32
    S = B // 2               # strips (2 blocks each)
    NT = NS // 128           # n-tiles
    G = S // SG              # strip groups
    CG = SG * 128            # columns per group
    WC = S // SC             # weight-prep chunks
    GPC = G // WC            # groups per weight chunk

    const_pool = ctx.enter_context(tc.tile_pool(name="const", bufs=1))
    wp = ctx.enter_context(tc.tile_pool(name="wp", bufs=2))
    wpsum = ctx.enter_context(tc.tile_pool(name="wpsum", bufs=2, space="PSUM"))
    xg_pool = ctx.enter_context(tc.tile_pool(name="xg", bufs=3))
    yg_pool = ctx.enter_context(tc.tile_pool(name="yg", bufs=3))
    xt_pool = ctx.enter_context(tc.tile_pool(name="xt", bufs=4))
    psx_pool = ctx.enter_context(tc.tile_pool(name="psx", bufs=3, space="PSUM"))
    psy_pool = ctx.enter_context(tc.tile_pool(name="psy", bufs=3, space="PSUM"))

    ident = const_pool.tile([128, 128], f32)
    make_identity(nc, ident)
    ident_t = const_pool.tile([128, 128], dt)
    nc.vector.tensor_copy(ident_t[:], ident[:])

    # masks [128, D]: partition p = 64*j + o, free = i
    tril_m = const_pool.tile([128, D], f32)   # 1 if i < o (strictly lower)
    diag_m = const_pool.tile([128, D], f32)   # 1 if i == o
    for h in range(2):
        tsl = tril_m[64 * h:64 * h + 64, :]
        nc.gpsimd.memset(tsl, 1.0)
        nc.gpsimd.affine_select(out=tsl, in_=tsl, compare_op=mybir.AluOpType.is_gt,
                                fill=0.0, base=0, pattern=[[-1, D]],
                                channel_multiplier=1)
        dsl = diag_m[64 * h:64 * h + 64, :]
        nc.gpsimd.memset(dsl, 0.0)
        nc.gpsimd.affine_select(out=dsl, in_=dsl,
                                compare_op=mybir.AluOpType.not_equal,
                                fill=1.0, base=0, pattern=[[-1, D]],
                                channel_multiplier=1)
    tril_mb = const_pool.tile([128, D], dt)
    diag_mb = const_pool.tile([128, D], dt)
    nc.vector.tensor_copy(tril_mb[:], tril_m[:])
    nc.vector.tensor_copy(diag_mb[:], diag_m[:])

    # all-resident transposed weights, stored directly as the block-diagonal
    # moving operand: lt[64j+i, s, 64j'+o] = (j==j') * L_{2s+j}[o, i].
    # The stage tile's off-diagonal quadrants are zeroed before the PE
    # transpose, so each transposed 128x128 tile IS block-diagonal as-is.
    lt = const_pool.tile([128, S, 128], dt)

    # diagonals (f32, 128 KB) resident up front — one contiguous DMA
    wd_sb = const_pool.tile([128, S], f32)
    nc.scalar.dma_start(wd_sb[:], wd_d[:, :])

    x_view = x_d.rearrange("(t p) c -> p t c", p=128)     # [128, NT, C]
    y_view = y_d.rearrange("(t p) c -> p t c", p=128)

    def prep_weight_chunk(c):
        """Build lt[:, c*SC:(c+1)*SC, :]."""
        # weight loads stay OFF the gpsimd ring (it is the x-load lifeline:
        # SWDGE is a single FIFO ring, so anything else on it stalls loads);
        # alternate the two HWDGE rings chunk by chunk
        weng = nc.sync if c % 2 == 0 else nc.scalar
        wr = wp.tile([128, SC, D], dt, tag="wr")
        weng.dma_start(wr[:], wt_d[:, c * SC:(c + 1) * SC, :])
        dexp = wp.tile([128, SC], dt, tag="dexp")
        nc.scalar.activation(dexp[:], wd_sb[:, c * SC:(c + 1) * SC],
                             mybir.ActivationFunctionType.Exp)
        shp = (128, SC, D)
        nc.gpsimd.tensor_tensor(wr[:], wr[:],
                                tril_mb[:, None, :].to_broadcast(shp),
                                op=mybir.AluOpType.mult)
        tmp = wp.tile(list(shp), dt, tag="wtmp")
        nc.gpsimd.tensor_tensor(tmp[:], diag_mb[:, None, :].to_broadcast(shp),
                                dexp[:, :, None].to_broadcast(shp),
                                op=mybir.AluOpType.mult)
        nc.vector.tensor_tensor(wr[:], wr[:], tmp[:], op=mybir.AluOpType.add)

        # Walrus requires transpose outputs at PSUM partition 0, so stage the
        # two 64x64 blocks on the diagonal of a 128x128 tile (off-diagonal
        # quadrants zeroed) and do one full transpose; the result is the
        # block-diagonal moving operand for this strip, stored whole.
        for sl in range(SC):
            stage = wp.tile([128, 128], dt, tag="wstage")
            nc.gpsimd.memset(stage[0:64, 64:128], 0.0)
            nc.gpsimd.memset(stage[64:128, 0:64], 0.0)
            nc.gpsimd.tensor_copy(stage[0:64, 0:64], wr[0:64, sl, :])
            nc.gpsimd.tensor_copy(stage[64:128, 64:128], wr[64:128, sl, :])
            pslt = wpsum.tile([128, 128], dt, tag="pslt")
            nc.tensor.matmul(pslt[:], lhsT=stage[:], rhs=ident_t[:],
                             is_transpose=True)
            s = c * SC + sl
            if sl % 2 == 0:
                nc.vector.tensor_copy(lt[:, s, :], pslt[:])
            else:
                nc.scalar.copy(lt[:, s, :], pslt[:])

    def do_group(g):
        xg = xg_pool.tile([128, NT, CG], dt, tag="xg")
        nc.gpsimd.dma_start(xg[:], x_view[:, :, g * CG:(g + 1) * CG])
        yg = yg_pool.tile([128, NT, CG], y_dt, tag="yg")
        for sl in range(SG):
            s = g * SG + sl
            # x chunks -> xT strip [128 (j,i), NS]
            psx = psx_pool.tile([128, NS], dt, tag="psx")
            for t in range(NT):
                nc.tensor.matmul(psx[:, t * 128:(t + 1) * 128],
                                 lhsT=xg[:, t, sl * 128:(sl + 1) * 128],
                                 rhs=ident_t[:], is_transpose=True,
                                 start=(t == 0), stop=(t == NT - 1))
            xt = xt_pool.tile([128, NS], dt, tag="xt")
            # PSUM->SBUF copies are DVE/ACT-only; alternate the pairing per
            # strip so neither engine eats both copy streams
            if s % 2 == 0:
                nc.vector.tensor_copy(xt[:], psx[:])
            else:
                nc.scalar.copy(xt[:], psx[:])
            # natural-output matmuls: stationary = xT chunk, moving = the
            # resident block-diag weight tile for this strip
            psy = psy_pool.tile([128, NS], f32, tag="psy")
            for t in range(NT):
                nc.tensor.matmul(psy[:, t * 128:(t + 1) * 128],
                                 lhsT=xt[:, t * 128:(t + 1) * 128],
                                 rhs=lt[:, s, :],
                                 start=(t == 0), stop=(t == NT - 1))
            if s % 2 == 0:
                nc.scalar.copy(yg[:, :, sl * 128:(sl + 1) * 128],
                               psy.rearrange("p (t c) -> p t c", c=128))
            else:
                nc.vector.tensor_copy(
                    yg[:, :, sl * 128:(sl + 1) * 128],
                    psy.rearrange("p (t c) -> p t c", c=128))
        nc.sync.dma_start(y_view[:, :, g * CG:(g + 1) * CG], yg[:])

    # interleave weight-chunk prep with the strip groups that consume it
    for _rep in range(repeat):
        for c in range(WC):
            prep_weight_chunk(c)
            for g in range(c * GPC, (c + 1) * GPC):
                do_group(g)


def build(NS=NS, B=B_FULL, SG=8, SC=32, dt_name="bf16", repeat=1):
    key = (NS, B, SG, SC, dt_name, repeat)
    if key in _built:
        return _built[key]
    import concourse.tile as tile
    import concourse.mybir as mybir
    from concourse import bacc

    f32 = mybir.dt.float32
    bf16 = mybir.dt.bfloat16
    C = B * D
    S = B // 2
    nc = bacc.Bacc("TRN2", target_bir_lowering=False, debug=False)
    x_d = nc.dram_tensor("x", [NS, C], bf16, kind="ExternalInput").ap()
    wt_d = nc.dram_tensor("wt", [128, S, D], bf16, kind="ExternalInput").ap()
    wd_d = nc.dram_tensor("wd", [128, S], f32, kind="ExternalInput").ap()
    y_d = nc.dram_tensor("y", [NS, C], bf16, kind="ExternalOutput").ap()
    with tile.TileContext(nc) as tc, ExitStack() as ctx:
        _body(ctx, tc, y_d, x_d, wt_d, wd_d, NS=NS, B=B, SG=SG, SC=SC,
              dt_name=dt_name, repeat=repeat, y_dt=bf16)
    nc.compile()
    _built[key] = nc
    return nc


def pack_weights(w32, B=B_FULL):
    """Pure layout permutation of W [B, D, D] for contiguous device DMA:
    wt[64j+o, s, i] = W[2s+j, o, i] (bf16); wd[64j+o, s] = W[2s+j, o, o]
    (f32)."""
    import ml_dtypes
    S = B // 2
    wt = np.ascontiguousarray(
        w32.reshape(S, 2, D, D).transpose(1, 2, 0, 3).reshape(128, S, D)
    ).astype(ml_dtypes.bfloat16)
    wdiag = w32[:, np.arange(D), np.arange(D)]            # [B, D]
    wd = np.ascontiguousarray(
        wdiag.reshape(S, 2, D).transpose(1, 2, 0).reshape(128, S),
        dtype=np.float32)
    return {"wt": wt, "wd": wd}


def _pin_compile_cache(extra=""):
    import hashlib
    with open(os.path.abspath(__file__), "rb") as f:
        h = hashlib.sha256(f.read() + extra.encode()).hexdigest()[:16]
    os.environ["NEURON_COMPILE_CACHE_URL"] = f"/tmp/neuron_cache_{h}"


def run(x, weights, trace=False, **build_kwargs):
    from concourse import bass_utils
    import ml_dtypes

    _pin_compile_cache()

    x = np.asarray(x)
    weights = np.asarray(weights)
    assert x.shape == (N_FULL, B_FULL * D), x.shape
    assert weights.shape == (B_FULL, D, D), weights.shape
    w32 = np.ascontiguousarray(weights, dtype=np.float32)
    wpack = pack_weights(w32)

    nc = build(**build_kwargs)
    xw = np.ascontiguousarray(x, dtype=ml_dtypes.bfloat16)
    in_maps = [{"x": xw[k * NS:(k + 1) * NS], **wpack} for k in range(NCORES)]
    res = bass_utils.run_bass_kernel_spmd(
        nc, in_maps, core_ids=list(range(NCORES)), trace=trace)
    y = np.concatenate([res.results[k]["y"] for k in range(NCORES)], axis=0)
    return np.asarray(y).astype(np.float32, copy=False), res


def kernel(x, weights):
    y, _ = run(x, weights)
    return y


# revision 37
# speedup vs baseline: 1.1150x; 1.1150x over previous
"""Trainium2 Bass kernel for nn_BatchedTrilLinear.

y[n, b*64:(b+1)*64] = x[n, b*64:(b+1)*64] @ L_b.T  for b in range(512),
with L_b = tril(W_b, -1) + diag(exp(diag(W_b))).

Sharding: data-parallel on N — each of the 8 cores processes a contiguous
512-row slice of x (zero-copy views) with the weights replicated.

Wire format: x and y cross HBM as bf16 (the matmul consumes bf16 anyway and
the 2e-2 gate dwarfs the bf16 output rounding), halving the dominant HBM
traffic. Weights ride as a host-side pure layout permutation of W:
  wt[64j+o, s, i] = W[2s+j, o, i]   (bf16, per-partition contiguous rows)
  wd[64j+o, s]    = W[2s+j, o, o]   (f32 diagonals for exact exp on device)
so the weight DMA uses large contiguous descriptors instead of 256 B strided
reads (which run ~5x below line rate). All math (tril mask, exp, transpose,
matmul) happens on device.

Per-core dataflow (natural-output matmul):
  - x loaded via SWDGE (gpsimd queue) in grouped HBM reads; y stored bf16
    via HWDGE on the SP queue; the two big DMA streams sit on different
    queues and overlap (measured IO-only: 64 MB in ~160 us ~ 400 GB/s).
  - weights: per chunk, load wt/wd, mask strictly-lower via tril mask
    (gpsimd), exp(wd) on ACT, insert diag via mask-mult + add, then stage
    each block pair on the diagonal of a zeroed 128x128 tile and
    PE-transpose, giving the all-resident block-diagonal moving operands
    lt[64j+i, s, 64j'+o] = (j==j') * L_{2s+j}[o, i]  (bf16, 8 MB),
    interleaved chunk-by-chunk with the strip groups that consume them.
  - per group of SG strips (strip = 128 columns = 2 blocks b0=2s, b1=2s+1),
    two phases so PE runs a transpose burst then a matmul burst instead of
    ping-ponging with the copy engines after every strip:
      phase 1, per strip PAIR: 8 PE transposes x chunks [128 n, 128 c] ->
        xT [128 (j,i), 2, 512 n] in one 1-bank bf16 PSUM tile; 1 fused
        PSUM->SBUF copy
      phase 2, per strip: 4 matmuls (stationary = xT chunk [128,128],
        moving = lt[:, s, :]) -> y lands in NATURAL [n, c] layout in a
        1-bank f32 PSUM tile; 1 copy (f32 PSUM -> bf16 group buffer)
      PSUM->SBUF copies are DVE/ACT-only (gpsimd has no PSUM port) and
      alternate between the two so neither carries both copy streams.
"""
import os
import sys
from contextlib import ExitStack

for _p in ("/opt/trn_rl_repo",):
    if os.path.isdir(_p) and _p not in sys.path:
        sys.path.insert(0, _p)

import numpy as np

N_FULL = 4096
B_FULL = 512
D = 64
NCORES = 8
NS = N_FULL // NCORES        # rows per core

_built = {}


def _body(ctx, tc, y_d, x_d, wt_d, wd_d, *, NS, B, SG, SC, dt_name, repeat=1,
          y_dt=None, io_only=False):
    import concourse.mybir as mybir
    from concourse.masks import make_identity

    nc = tc.nc
    f32 = mybir.dt.float32
    dt = {"bf16": mybir.dt.bfloat16, "f32r": mybir.dt.float32r}[dt_name]
    if y_dt is None:
        y_dt = f32
    S = B // 2               # strips (2 blocks each)
    NT = NS // 128           # n-tiles
    G = S // SG              # strip groups
    CG = SG * 128            # columns per group
    WC = S // SC             # weight-prep chunks
    GPC = G // WC            # groups per weight chunk

    const_pool = ctx.enter_context(tc.tile_pool(name="const", bufs=1))
    wp = ctx.enter_context(tc.tile_pool(name="wp", bufs=2))
    wpsum = ctx.enter_context(tc.tile_pool(name="wpsum", bufs=1, space="PSUM"))
    xg_pool = ctx.enter_context(tc.tile_pool(name="xg", bufs=3))
    yg_pool = ctx.enter_context(tc.tile_pool(name="yg", bufs=3))
    xt_pool = ctx.enter_context(tc.tile_pool(name="xt", bufs=SG // 2 + 2))
    psx_pool = ctx.enter_context(tc.tile_pool(name="psx", bufs=3, space="PSUM"))
    psy_pool = ctx.enter_context(tc.tile_pool(name="psy", bufs=4, space="PSUM"))

    ident = const_pool.tile([128, 128], f32)
    make_identity(nc, ident)
    ident_t = const_pool.tile([128, 128], dt)
    nc.vector.tensor_copy(ident_t[:], ident[:])

    # masks [128, D]: partition p = 64*j + o, free = i
    tril_m = const_pool.tile([128, D], f32)   # 1 if i < o (strictly lower)
    diag_m = const_pool.tile([128, D], f32)   # 1 if i == o
    for h in range(2):
        tsl = tril_m[64 * h:64 * h + 64, :]
        nc.gpsimd.memset(tsl, 1.0)
        nc.gpsimd.affine_select(out=tsl, in_=tsl, compare_op=mybir.AluOpType.is_gt,
                                fill=0.0, base=0, pattern=[[-1, D]],
                                channel_multiplier=1)
        dsl = diag_m[64 * h:64 * h + 64, :]
        nc.gpsimd.memset(dsl, 0.0)
        nc.gpsimd.affine_select(out=dsl, in_=dsl,
                                compare_op=mybir.AluOpType.not_equal,
                                fill=1.0, base=0, pattern=[[-1, D]],
                                channel_multiplier=1)
    tril_mb = const_pool.tile([128, D], dt)
    diag_mb = const_pool.tile([128, D], dt)
    nc.vector.tensor_copy(tril_mb[:], tril_m[:])
    nc.vector.tensor_copy(diag_mb[:], diag_m[:])

    # all-resident transposed weights, stored directly as the block-diagonal
    # moving operand: lt[64j+i, s, 64j'+o] = (j==j') * L_{2s+j}[o, i].
    # The stage tile's off-diagonal quadrants are zeroed before the PE
    # transpose, so each transposed 128x128 tile IS block-diagonal as-is.
    lt = const_pool.tile([128, S, 128], dt)

    # diagonals (f32, 128 KB) resident up front — one contiguous DMA
    wd_sb = const_pool.tile([128, S], f32)
    nc.scalar.dma_start(wd_sb[:], wd_d[:, :])

    x_view = x_d.rearrange("(t p) c -> p t c", p=128)     # [128, NT, C]
    y_view = y_d.rearrange("(t p) c -> p t c", p=128)

    def prep_weight_chunk(c):
        """Build lt[:, c*SC:(c+1)*SC, :]."""
        # weight loads stay OFF the gpsimd ring (it is the x-load lifeline:
        # SWDGE is a single FIFO ring, so anything else on it stalls loads);
        # alternate the two HWDGE rings chunk by chunk
        weng = nc.sync if c % 2 == 0 else nc.scalar
        wr = wp.tile([128, SC, D], dt, tag="wr")
        weng.dma_start(wr[:], wt_d[:, c * SC:(c + 1) * SC, :])
        dexp = wp.tile([128, SC], dt, tag="dexp")
        nc.scalar.activation(dexp[:], wd_sb[:, c * SC:(c + 1) * SC],
                             mybir.ActivationFunctionType.Exp)
        shp = (128, SC, D)
        nc.gpsimd.tensor_tensor(wr[:], wr[:],
                                tril_mb[:, None, :].to_broadcast(shp),
                                op=mybir.AluOpType.mult)
        tmp = wp.tile(list(shp), dt, tag="wtmp")
        nc.gpsimd.tensor_tensor(tmp[:], diag_mb[:, None, :].to_broadcast(shp),
                                dexp[:, :, None].to_broadcast(shp),
                                op=mybir.AluOpType.mult)
        nc.vector.tensor_tensor(wr[:], wr[:], tmp[:], op=mybir.AluOpType.add)

        # Walrus requires transpose outputs at PSUM partition 0, so stage the
        # two 64x64 blocks on the diagonal of a 128x128 tile (off-diagonal
        # quadrants zeroed) and do one full transpose; the result is the
        # block-diagonal moving operand for this strip, stored whole.
        for sl in range(SC):
            stage = wp.tile([128, 128], dt, tag="wstage")
            nc.gpsimd.memset(stage[0:64, 64:128], 0.0)
            nc.gpsimd.memset(stage[64:128, 0:64], 0.0)
            nc.gpsimd.tensor_copy(stage[0:64, 0:64], wr[0:64, sl, :])
            nc.gpsimd.tensor_copy(stage[64:128, 64:128], wr[64:128, sl, :])
            pslt = wpsum.tile([128, 128], dt, tag="pslt")
            nc.tensor.matmul(pslt[:], lhsT=stage[:], rhs=ident_t[:],
                             is_transpose=True)
            s = c * SC + sl
            if sl % 2 == 0:
                nc.vector.tensor_copy(lt[:, s, :], pslt[:])
            else:
                nc.scalar.copy(lt[:, s, :], pslt[:])

    # Phase 1: x load + all transposes + fused-pair PSUM->SBUF copies for a
    # group; phase 2: all matmuls + y copies. Phase 1 of group g+1 is
    # emitted BEFORE phase 2 of group g (software pipeline) so PE swings
    # between a transpose burst and a matmul burst with no copy round-trip
    # stalls in between — which also keeps PE busy enough to stay out of
    # the HAM cold-clock (1.2 GHz) state.
    def phase1(g):
        xg = xg_pool.tile([128, NT, CG], dt, tag="xg")
        nc.gpsimd.dma_start(xg[:], x_view[:, :, g * CG:(g + 1) * CG])
        xts = []
        for sl in range(0, SG, 2):
            psx = psx_pool.tile([128, 2, NS], dt, tag="psx")
            for q in range(2):
                for t in range(NT):
                    nc.tensor.matmul(psx[:, q, t * 128:(t + 1) * 128],
                                     lhsT=xg[:, t, (sl + q) * 128:(sl + q + 1) * 128],
                                     rhs=ident_t[:], is_transpose=True,
                                     start=(q == 0 and t == 0),
                                     stop=(q == 1 and t == NT - 1))
            xt = xt_pool.tile([128, 2, NS], dt, tag="xt")
            if (sl // 2) % 2 == 0:
                nc.vector.tensor_copy(xt[:], psx[:])
            else:
                nc.scalar.copy(xt[:], psx[:])
            xts.append(xt)
        return xts

    def phase2(g, xts):
        yg = yg_pool.tile([128, NT, CG], y_dt, tag="yg")
        for sl in range(SG):
            s = g * SG + sl
            xt = xts[sl // 2]
            q = sl & 1
            # natural-output matmuls: stationary = xT chunk, moving = the
            # resident block-diag weight tile for this strip. NB: keep each
            # matmul accumulation group within ONE 2 KB PSUM bank — a
            # psy tile spanning two banks passes CoreSim but returns
            # garbage on hardware.
            psy = psy_pool.tile([128, NS], f32, tag="psy")
            for t in range(NT):
                nc.tensor.matmul(psy[:, t * 128:(t + 1) * 128],
                                 lhsT=xt[:, q, t * 128:(t + 1) * 128],
                                 rhs=lt[:, s, :],
                                 start=(t == 0), stop=(t == NT - 1))
            if sl % 2 == 0:
                nc.scalar.copy(yg[:, :, sl * 128:(sl + 1) * 128],
                               psy.rearrange("p (t c) -> p t c", c=128))
            else:
                nc.vector.tensor_copy(
                    yg[:, :, sl * 128:(sl + 1) * 128],
                    psy.rearrange("p (t c) -> p t c", c=128))
        nc.sync.dma_start(y_view[:, :, g * CG:(g + 1) * CG], yg[:])

    if io_only:
        # DMA-ablation build: stream x in and store a dummy yg, no compute.
        # NOT functionally correct — timing probe only.
        ygd = yg_pool.tile([128, NT, CG], y_dt, tag="ygd")
        nc.gpsimd.memset(ygd[:], 0.0)
        for _rep in range(repeat):
            for g in range(G):
                xg = xg_pool.tile([128, NT, CG], dt, tag="xg")
                nc.gpsimd.dma_start(xg[:], x_view[:, :, g * CG:(g + 1) * CG])
                nc.sync.dma_start(y_view[:, :, g * CG:(g + 1) * CG], ygd[:])
        return

    # interleave weight-chunk prep with the strip groups that consume it
    for _rep in range(repeat):
        for c in range(WC):
            prep_weight_chunk(c)
            for g in range(c * GPC, (c + 1) * GPC):
                phase2(g, phase1(g))


def build(NS=NS, B=B_FULL, SG=16, SC=32, dt_name="bf16", repeat=1,
          io_only=False):
    key = (NS, B, SG, SC, dt_name, repeat, io_only)
    if key in _built:
        return _built[key]
    import concourse.tile as tile
    import concourse.mybir as mybir
    from concourse import bacc

    f32 = mybir.dt.float32
    bf16 = mybir.dt.bfloat16
    C = B * D
    S = B // 2
    nc = bacc.Bacc("TRN2", target_bir_lowering=False, debug=False)
    x_d = nc.dram_tensor("x", [NS, C], bf16, kind="ExternalInput").ap()
    wt_d = nc.dram_tensor("wt", [128, S, D], bf16, kind="ExternalInput").ap()
    wd_d = nc.dram_tensor("wd", [128, S], f32, kind="ExternalInput").ap()
    y_d = nc.dram_tensor("y", [NS, C], bf16, kind="ExternalOutput").ap()
    with tile.TileContext(nc) as tc, ExitStack() as ctx:
        _body(ctx, tc, y_d, x_d, wt_d, wd_d, NS=NS, B=B, SG=SG, SC=SC,
              dt_name=dt_name, repeat=repeat, y_dt=bf16, io_only=io_only)
    nc.compile()
    _built[key] = nc
    return nc


def pack_weights(w32, B=B_FULL):
    """Pure layout permutation of W [B, D, D] for contiguous device DMA:
    wt[64j+o, s, i] = W[2s+j, o, i] (bf16); wd[64j+o, s] = W[2s+j, o, o]
    (f32)."""
    import ml_dtypes
    S = B // 2
    wt = np.ascontiguousarray(
        w32.reshape(S, 2, D, D).transpose(1, 2, 0, 3).reshape(128, S, D)
    ).astype(ml_dtypes.bfloat16)
    wdiag = w32[:, np.arange(D), np.arange(D)]            # [B, D]
    wd = np.ascontiguousarray(
        wdiag.reshape(S, 2, D).transpose(1, 2, 0).reshape(128, S),
        dtype=np.float32)
    return {"wt": wt, "wd": wd}


def _pin_compile_cache(extra=""):
    import hashlib
    with open(os.path.abspath(__file__), "rb") as f:
        h = hashlib.sha256(f.read() + extra.encode()).hexdigest()[:16]
    os.environ["NEURON_COMPILE_CACHE_URL"] = f"/tmp/neuron_cache_{h}"


def run(x, weights, trace=False, **build_kwargs):
    from concourse import bass_utils
    import ml_dtypes

    _pin_compile_cache()

    x = np.asarray(x)
    weights = np.asarray(weights)
    assert x.shape == (N_FULL, B_FULL * D), x.shape
    assert weights.shape == (B_FULL, D, D), weights.shape
    w32 = np.ascontiguousarray(weights, dtype=np.float32)
    wpack = pack_weights(w32)

    nc = build(**build_kwargs)
    xw = np.ascontiguousarray(x, dtype=ml_dtypes.bfloat16)
    in_maps = [{"x": xw[k * NS:(k + 1) * NS], **wpack} for k in range(NCORES)]
    res = bass_utils.run_bass_kernel_spmd(
        nc, in_maps, core_ids=list(range(NCORES)), trace=trace)
    y = np.concatenate([res.results[k]["y"] for k in range(NCORES)], axis=0)
    return np.asarray(y).astype(np.float32, copy=False), res


def kernel(x, weights):
    y, _ = run(x, weights)
    return y


# revision 43
# speedup vs baseline: 1.1767x; 1.0553x over previous
"""Trainium2 Bass kernel for nn_BatchedTrilLinear.

y[n, b*64:(b+1)*64] = x[n, b*64:(b+1)*64] @ L_b.T  for b in range(512),
with L_b = tril(W_b, -1) + diag(exp(diag(W_b))).

Sharding: data-parallel on N — each of the 8 cores processes a contiguous
512-row slice of x (zero-copy views) with the weights replicated.

Wire format: x and y cross HBM as bf16 (the matmul consumes bf16 anyway and
the 2e-2 gate dwarfs the bf16 output rounding), halving the dominant HBM
traffic. Weights ride as a host-side pure layout permutation of W:
  wt[64j+o, s, i] = W[2s+j, o, i]   (bf16, per-partition contiguous rows)
  wd[64j+o, s]    = W[2s+j, o, o]   (f32 diagonals for exact exp on device)
so the weight DMA uses large contiguous descriptors instead of 256 B strided
reads (which run ~5x below line rate). All math (tril mask, exp, transpose,
matmul) happens on device.

Per-core dataflow (natural-output matmul):
  - x loaded via SWDGE (gpsimd queue) in grouped HBM reads; y stored bf16
    via HWDGE on the SP queue; the two big DMA streams sit on different
    queues and overlap (measured IO-only: 64 MB in ~160 us ~ 400 GB/s).
  - weights: per chunk, load wt/wd, mask strictly-lower via tril mask
    (gpsimd), exp(wd) on ACT, insert diag via mask-mult + add, then stage
    each block pair on the diagonal of a zeroed 128x128 tile and
    PE-transpose, giving the all-resident block-diagonal moving operands
    lt[64j+i, s, 64j'+o] = (j==j') * L_{2s+j}[o, i]  (bf16, 8 MB),
    interleaved chunk-by-chunk with the strip groups that consume them.
  - per group of SG strips (strip = 128 columns = 2 blocks b0=2s, b1=2s+1),
    two phases so PE runs a transpose burst then a matmul burst instead of
    ping-ponging with the copy engines after every strip:
      phase 1, per strip PAIR: 8 PE transposes x chunks [128 n, 128 c] ->
        xT [128 (j,i), 2, 512 n] in one 1-bank bf16 PSUM tile; 1 fused
        PSUM->SBUF copy
      phase 2, per strip: 4 matmuls (stationary = xT chunk [128,128],
        moving = lt[:, s, :]) -> y lands in NATURAL [n, c] layout in a
        1-bank f32 PSUM tile; 1 copy (f32 PSUM -> bf16 group buffer)
      PSUM->SBUF copies are DVE/ACT-only (gpsimd has no PSUM port) and
      alternate between the two so neither carries both copy streams.
"""
import os
import sys
from contextlib import ExitStack

for _p in ("/opt/trn_rl_repo",):
    if os.path.isdir(_p) and _p not in sys.path:
        sys.path.insert(0, _p)

import numpy as np

N_FULL = 4096
B_FULL = 512
D = 64
NCORES = 8
NS = N_FULL // NCORES        # rows per core

_built = {}


def _body(ctx, tc, y_d, x_d, wt_d, wd_d, *, NS, B, SG, SC, dt_name, repeat=1,
          y_dt=None, io_only=False):
    import concourse.mybir as mybir
    from concourse.masks import make_identity

    nc = tc.nc
    f32 = mybir.dt.float32
    dt = {"bf16": mybir.dt.bfloat16, "f32r": mybir.dt.float32r}[dt_name]
    if y_dt is None:
        y_dt = f32
    S = B // 2               # strips (2 blocks each)
    NT = NS // 128           # n-tiles
    G = S // SG              # strip groups
    CG = SG * 128            # columns per group
    WC = S // SC             # weight-prep chunks
    GPC = G // WC            # groups per weight chunk

    const_pool = ctx.enter_context(tc.tile_pool(name="const", bufs=1))
    wp = ctx.enter_context(tc.tile_pool(name="wp", bufs=2))
    wpsum = ctx.enter_context(tc.tile_pool(name="wpsum", bufs=1, space="PSUM"))
    xg_pool = ctx.enter_context(tc.tile_pool(name="xg", bufs=4))
    yg_pool = ctx.enter_context(tc.tile_pool(name="yg", bufs=2))
    xt_pool = ctx.enter_context(tc.tile_pool(name="xt", bufs=SG // 2 + 2))
    psx_pool = ctx.enter_context(tc.tile_pool(name="psx", bufs=3, space="PSUM"))
    psy_pool = ctx.enter_context(tc.tile_pool(name="psy", bufs=4, space="PSUM"))

    ident = const_pool.tile([128, 128], f32)
    make_identity(nc, ident)
    ident_t = const_pool.tile([128, 128], dt)
    nc.vector.tensor_copy(ident_t[:], ident[:])

    # masks [128, D]: partition p = 64*j + o, free = i
    tril_m = const_pool.tile([128, D], f32)   # 1 if i < o (strictly lower)
    diag_m = const_pool.tile([128, D], f32)   # 1 if i == o
    for h in range(2):
        tsl = tril_m[64 * h:64 * h + 64, :]
        nc.gpsimd.memset(tsl, 1.0)
        nc.gpsimd.affine_select(out=tsl, in_=tsl, compare_op=mybir.AluOpType.is_gt,
                                fill=0.0, base=0, pattern=[[-1, D]],
                                channel_multiplier=1)
        dsl = diag_m[64 * h:64 * h + 64, :]
        nc.gpsimd.memset(dsl, 0.0)
        nc.gpsimd.affine_select(out=dsl, in_=dsl,
                                compare_op=mybir.AluOpType.not_equal,
                                fill=1.0, base=0, pattern=[[-1, D]],
                                channel_multiplier=1)
    tril_mb = const_pool.tile([128, D], dt)
    diag_mb = const_pool.tile([128, D], dt)
    nc.vector.tensor_copy(tril_mb[:], tril_m[:])
    nc.vector.tensor_copy(diag_mb[:], diag_m[:])

    # all-resident transposed weights, stored directly as the block-diagonal
    # moving operand: lt[64j+i, s, 64j'+o] = (j==j') * L_{2s+j}[o, i].
    # The stage tile's off-diagonal quadrants are zeroed before the PE
    # transpose, so each transposed 128x128 tile IS block-diagonal as-is.
    lt = const_pool.tile([128, S, 128], dt)

    # diagonals (f32, 128 KB) resident up front — one contiguous DMA
    wd_sb = const_pool.tile([128, S], f32)
    nc.scalar.dma_start(wd_sb[:], wd_d[:, :])

    x_view = x_d.rearrange("(t p) c -> p t c", p=128)     # [128, NT, C]
    y_view = y_d.rearrange("(t p) c -> p t c", p=128)

    def prep_weight_chunk(c):
        """Build lt[:, c*SC:(c+1)*SC, :]."""
        # weight loads stay OFF the gpsimd ring (it is the x-load lifeline:
        # SWDGE is a single FIFO ring, so anything else on it stalls loads);
        # alternate the two HWDGE rings chunk by chunk
        weng = nc.sync if c % 2 == 0 else nc.scalar
        wr = wp.tile([128, SC, D], dt, tag="wr")
        weng.dma_start(wr[:], wt_d[:, c * SC:(c + 1) * SC, :])
        dexp = wp.tile([128, SC], dt, tag="dexp")
        nc.scalar.activation(dexp[:], wd_sb[:, c * SC:(c + 1) * SC],
                             mybir.ActivationFunctionType.Exp)
        shp = (128, SC, D)
        nc.gpsimd.tensor_tensor(wr[:], wr[:],
                                tril_mb[:, None, :].to_broadcast(shp),
                                op=mybir.AluOpType.mult)
        tmp = wp.tile(list(shp), dt, tag="wtmp")
        nc.gpsimd.tensor_tensor(tmp[:], diag_mb[:, None, :].to_broadcast(shp),
                                dexp[:, :, None].to_broadcast(shp),
                                op=mybir.AluOpType.mult)
        nc.vector.tensor_tensor(wr[:], wr[:], tmp[:], op=mybir.AluOpType.add)

        # Walrus requires transpose outputs at PSUM partition 0, so stage the
        # two 64x64 blocks on the diagonal of a 128x128 tile (off-diagonal
        # quadrants zeroed) and do one full transpose; the result is the
        # block-diagonal moving operand for this strip, stored whole.
        for sl in range(SC):
            stage = wp.tile([128, 128], dt, tag="wstage")
            nc.gpsimd.memset(stage[0:64, 64:128], 0.0)
            nc.gpsimd.memset(stage[64:128, 0:64], 0.0)
            nc.gpsimd.tensor_copy(stage[0:64, 0:64], wr[0:64, sl, :])
            nc.gpsimd.tensor_copy(stage[64:128, 64:128], wr[64:128, sl, :])
            pslt = wpsum.tile([128, 128], dt, tag="pslt")
            nc.tensor.matmul(pslt[:], lhsT=stage[:], rhs=ident_t[:],
                             is_transpose=True)
            s = c * SC + sl
            # lt copies all on DVE: ACT is the busier copy engine
            nc.vector.tensor_copy(lt[:, s, :], pslt[:])

    # Phase 1: x load + all transposes + fused-pair PSUM->SBUF copies for a
    # group; phase 2: all matmuls + y copies. Phase 1 of group g+1 is
    # emitted BEFORE phase 2 of group g (software pipeline) so PE swings
    # between a transpose burst and a matmul burst with no copy round-trip
    # stalls in between — which also keeps PE busy enough to stay out of
    # the HAM cold-clock (1.2 GHz) state.
    def phase1(g):
        xg = xg_pool.tile([128, NT, CG], dt, tag="xg")
        # alternate x loads between the SWDGE ring and the SP HWDGE ring:
        # a single queue serializes the 2 MB loads behind each other and
        # the machine stalls waiting for the next group (seen as ~4 us
        # all-engine gaps per group in the timeline)
        xeng = nc.gpsimd if g % 2 == 0 else nc.sync
        xeng.dma_start(xg[:], x_view[:, :, g * CG:(g + 1) * CG])
        xts = []
        for sl in range(0, SG, 2):
            psx = psx_pool.tile([128, 2, NS], dt, tag="psx")
            for q in range(2):
                for t in range(NT):
                    nc.tensor.matmul(psx[:, q, t * 128:(t + 1) * 128],
                                     lhsT=xg[:, t, (sl + q) * 128:(sl + q + 1) * 128],
                                     rhs=ident_t[:], is_transpose=True,
                                     start=(q == 0 and t == 0),
                                     stop=(q == 1 and t == NT - 1))
            xt = xt_pool.tile([128, 2, NS], dt, tag="xt")
            if (sl // 2) % 2 == 0:
                nc.vector.tensor_copy(xt[:], psx[:])
            else:
                nc.scalar.copy(xt[:], psx[:])
            xts.append(xt)
        return xts

    def phase2(g, xts):
        yg = yg_pool.tile([128, NT, CG], y_dt, tag="yg")
        for sl in range(SG):
            s = g * SG + sl
            xt = xts[sl // 2]
            q = sl & 1
            # natural-output matmuls: stationary = xT chunk, moving = the
            # resident block-diag weight tile for this strip. NB: keep each
            # matmul accumulation group within ONE 2 KB PSUM bank — a
            # psy tile spanning two banks passes CoreSim but returns
            # garbage on hardware.
            psy = psy_pool.tile([128, NS], f32, tag="psy")
            for t in range(NT):
                nc.tensor.matmul(psy[:, t * 128:(t + 1) * 128],
                                 lhsT=xt[:, q, t * 128:(t + 1) * 128],
                                 rhs=lt[:, s, :],
                                 start=(t == 0), stop=(t == NT - 1))
            if sl % 2 == 0:
                nc.scalar.copy(yg[:, :, sl * 128:(sl + 1) * 128],
                               psy.rearrange("p (t c) -> p t c", c=128))
            else:
                nc.vector.tensor_copy(
                    yg[:, :, sl * 128:(sl + 1) * 128],
                    psy.rearrange("p (t c) -> p t c", c=128))
        nc.sync.dma_start(y_view[:, :, g * CG:(g + 1) * CG], yg[:])

    if io_only:
        # DMA-ablation build: stream x in and store a dummy yg, no compute.
        # NOT functionally correct — timing probe only.
        ygd = yg_pool.tile([128, NT, CG], y_dt, tag="ygd")
        nc.gpsimd.memset(ygd[:], 0.0)
        for _rep in range(repeat):
            for g in range(G):
                xg = xg_pool.tile([128, NT, CG], dt, tag="xg")
                nc.gpsimd.dma_start(xg[:], x_view[:, :, g * CG:(g + 1) * CG])
                nc.sync.dma_start(y_view[:, :, g * CG:(g + 1) * CG], ygd[:])
        return

    # interleave weight-chunk prep with the strip groups that consume it,
    # prefetching each chunk's prep one chunk ahead so its DMA + mask +
    # transpose chain never gates the first strips of its groups
    for _rep in range(repeat):
        prep_weight_chunk(0)
        for c in range(WC):
            if c + 1 < WC:
                prep_weight_chunk(c + 1)
            for g in range(c * GPC, (c + 1) * GPC):
                phase2(g, phase1(g))


def build(NS=NS, B=B_FULL, SG=16, SC=32, dt_name="bf16", repeat=1,
          io_only=False):
    key = (NS, B, SG, SC, dt_name, repeat, io_only)
    if key in _built:
        return _built[key]
    import concourse.tile as tile
    import concourse.mybir as mybir
    from concourse import bacc

    f32 = mybir.dt.float32
    bf16 = mybir.dt.bfloat16
    C = B * D
    S = B // 2
    nc = bacc.Bacc("TRN2", target_bir_lowering=False, debug=False)
    x_d = nc.dram_tensor("x", [NS, C], bf16, kind="ExternalInput").ap()
    wt_d = nc.dram_tensor("wt", [128, S, D], bf16, kind="ExternalInput").ap()
    wd_d = nc.dram_tensor("wd", [128, S], f32, kind="ExternalInput").ap()
    y_d = nc.dram_tensor("y", [NS, C], bf16, kind="ExternalOutput").ap()
    with tile.TileContext(nc) as tc, ExitStack() as ctx:
        _body(ctx, tc, y_d, x_d, wt_d, wd_d, NS=NS, B=B, SG=SG, SC=SC,
              dt_name=dt_name, repeat=repeat, y_dt=bf16, io_only=io_only)
    nc.compile()
    _built[key] = nc
    return nc


def pack_weights(w32, B=B_FULL):
    """Pure layout permutation of W [B, D, D] for contiguous device DMA:
    wt[64j+o, s, i] = W[2s+j, o, i] (bf16); wd[64j+o, s] = W[2s+j, o, o]
    (f32)."""
    import ml_dtypes
    S = B // 2
    wt = np.ascontiguousarray(
        w32.reshape(S, 2, D, D).transpose(1, 2, 0, 3).reshape(128, S, D)
    ).astype(ml_dtypes.bfloat16)
    wdiag = w32[:, np.arange(D), np.arange(D)]            # [B, D]
    wd = np.ascontiguousarray(
        wdiag.reshape(S, 2, D).transpose(1, 2, 0).reshape(128, S),
        dtype=np.float32)
    return {"wt": wt, "wd": wd}


def _pin_compile_cache(extra=""):
    import hashlib
    with open(os.path.abspath(__file__), "rb") as f:
        h = hashlib.sha256(f.read() + extra.encode()).hexdigest()[:16]
    os.environ["NEURON_COMPILE_CACHE_URL"] = f"/tmp/neuron_cache_{h}"


def run(x, weights, trace=False, **build_kwargs):
    from concourse import bass_utils
    import ml_dtypes

    _pin_compile_cache()

    x = np.asarray(x)
    weights = np.asarray(weights)
    assert x.shape == (N_FULL, B_FULL * D), x.shape
    assert weights.shape == (B_FULL, D, D), weights.shape
    w32 = np.ascontiguousarray(weights, dtype=np.float32)
    wpack = pack_weights(w32)

    nc = build(**build_kwargs)
    xw = np.ascontiguousarray(x, dtype=ml_dtypes.bfloat16)
    in_maps = [{"x": xw[k * NS:(k + 1) * NS], **wpack} for k in range(NCORES)]
    res = bass_utils.run_bass_kernel_spmd(
        nc, in_maps, core_ids=list(range(NCORES)), trace=trace)
    y = np.concatenate([res.results[k]["y"] for k in range(NCORES)], axis=0)
    return np.asarray(y).astype(np.float32, copy=False), res


def kernel(x, weights):
    y, _ = run(x, weights)
    return y
